# revision 19
# baseline (speedup 1.0000x reference)
"""BinaryContrastiveLoss Trainium2 kernel.

Contract: kernel(**inputs) takes the FULL unsharded inputs
  features:       [8, 4096, 128] float32
  positive_index: [8, 4096, 16]  int64
  negative_index: [8, 4096, 32]  int64
and returns the scalar loss (np.float32), matching reference().

Sharding: data-parallel over the batch dim B=8 -> 8 NeuronCores, one
batch element per core.  All gathers are local to a batch element.
Each core computes S_b = sum_n sum_p softplus(pos_dot - ln(denom_n));
host combines: loss = mean_b( -S_b / (P*N) ).

Device algorithm per core (N=4096 tokens, D=128, K=48 pairs/token):
  phase 1: load features, L2-normalize, cast to bf16; keep resident in
           SBUF and also write a bf16 table to DRAM (gather source).
  phase 2: per 128-token tile: one indirect DMA gathers the 48 target
           rows per token ([128, 48, 128] bf16), dots via per-k
           tensor_tensor_reduce on DVE, then exp/ln/softplus on ACT.
  phase 3: reduce per-tile losses to a single scalar (PE ones-matmul
           for the partition reduction) and DMA it out.
"""

import sys

if "/opt/trn_rl_repo" not in sys.path:
    sys.path.insert(0, "/opt/trn_rl_repo")

import numpy as np

B, N, D, P, Q = 8, 4096, 128, 16, 32
K = P + Q
TILE = 128
NT = N // TILE
KC = 8                 # k's per dma_gather call (1024 idx ring limit)
NCALL = K // KC        # 6 gather calls per tile

_CACHE = {}


def build_program():
    if "nc" in _CACHE:
        return _CACHE["nc"]

    from concourse import bacc, bass, mybir, tile

    f32 = mybir.dt.float32
    bf16 = mybir.dt.bfloat16
    i16 = mybir.dt.int16
    AF = mybir.ActivationFunctionType
    ALU = mybir.AluOpType

    nc = bacc.Bacc(None, target_bir_lowering=False, num_swdge_queues=4)
    feats = nc.dram_tensor("features", [N, D], f32, kind="ExternalInput")
    # wrapped int16 gather indices; 6 calls x 1024 idxs per tile (the
    # SWDGE descriptor ring caps a single dma_gather at ~1024 indices)
    idxw = nc.dram_tensor(
        "idxw", [NT, NCALL, 128, KC * TILE // 16], i16, kind="ExternalInput"
    )
    out = nc.dram_tensor("out", [1, 1], f32, kind="ExternalOutput")
    table = nc.dram_tensor("table", [N, D], bf16)

    with tile.TileContext(nc) as tc:
        with (
            tc.tile_pool(name="const", bufs=1) as cpool,
            tc.tile_pool(name="work", bufs=3) as work,
            tc.tile_pool(name="gather", bufs=6) as gpool,
            tc.tile_pool(name="psum", bufs=1, space="PSUM") as psum,
        ):
            # resident normalized feats — one tile per 128-token block so
            # dependency tracking stays slice-granular (a single big tile
            # accumulates too many sync waits per instruction).
            fnorm = [
                cpool.tile([TILE, D], bf16, tag=f"fn{t}", name=f"fnorm{t}")
                for t in range(NT)
            ]
            tlosses = cpool.tile([TILE, NT], f32)     # per-tile loss columns
            ones = cpool.tile([TILE, 1], f32)
            nc.vector.memset(ones[:], 1.0)

            # ---- phase 1: normalize ----
            # one strided DMA loads all features ([128, NT, D]); per-tile
            # chains then run without per-tile load latency, so the table
            # (gather source) is complete as early as possible.
            ft_all = cpool.tile([TILE, NT, D], f32)
            nc.sync.dma_start(
                out=ft_all[:], in_=feats[:].rearrange("(t p) d -> p t d", p=TILE)
            )
            # one-shot wide normalization stats over all 32 tiles
            sq_all = cpool.tile([TILE, NT, D], f32)
            nc.vector.tensor_tensor(
                out=sq_all[:], in0=ft_all[:], in1=ft_all[:], op=ALU.mult
            )
            ss_all = cpool.tile([TILE, NT], f32)
            nc.vector.tensor_reduce(
                out=ss_all[:], in_=sq_all[:], axis=mybir.AxisListType.X, op=ALU.add
            )
            rs_all = cpool.tile([TILE, NT], f32)
            nc.vector.reciprocal(rs_all[:], ss_all[:])
            ri_all = cpool.tile([TILE, NT], f32)
            nc.scalar.sqrt(ri_all[:], rs_all[:])  # 1/||f|| = sqrt(1/ss)
            for t in range(NT):
                rows = slice(t * TILE, (t + 1) * TILE)
                nc.scalar.mul(fnorm[t][:], ft_all[:, t, :], ri_all[:, t : t + 1])
                nc.sync.dma_start(out=table[rows, :], in_=fnorm[t][:])

            # ---- phase 2: gather + dots + loss ----
            for t in range(NT):
                rows = slice(t * TILE, (t + 1) * TILE)
                g = gpool.tile([TILE, K, D], bf16, tag="g")
                for c in range(NCALL):
                    it = work.tile(
                        [128, KC * TILE // 16], i16, tag=f"it{c}", name=f"it_{t}_{c}"
                    )
                    nc.sync.dma_start(out=it[:], in_=idxw[t, c])
                    nc.gpsimd.dma_gather(
                        out_ap=g[:, c * KC : (c + 1) * KC, :],
                        in_ap=table[:],
                        idxs_ap=it[:],
                        num_idxs=KC * TILE,
                        num_idxs_reg=KC * TILE,
                        elem_size=D,
                        queue_num=(t * NCALL + c) % 4,
                    )
                R = work.tile([TILE, K], f32, tag="R")
                # dots per gather call (KC k's at a time) for finer overlap
                # with the SWDGE descriptor generation; staged tree reduce
                # keeps most elements in the DVE 2x path (plain tensor_reduce
                # runs at 1x only).
                prod = work.tile([TILE, KC, D], bf16, tag="prod")
                half = work.tile([TILE, KC, D // 2], bf16, tag="half")
                quar = work.tile([TILE, KC, D // 4], bf16, tag="quar")
                for c in range(NCALL):
                    ks = slice(c * KC, (c + 1) * KC)
                    nc.vector.tensor_tensor(
                        out=prod[:],
                        in0=g[:, ks, :],
                        in1=fnorm[t][:].unsqueeze(1).broadcast_to([TILE, KC, D]),
                        op=ALU.mult,
                    )
                    nc.vector.tensor_tensor(
                        out=half[:],
                        in0=prod[:, :, 0 : D // 2],
                        in1=prod[:, :, D // 2 : D],
                        op=ALU.add,
                    )
                    nc.vector.tensor_tensor(
                        out=quar[:],
                        in0=half[:, :, 0 : D // 4],
                        in1=half[:, :, D // 4 : D // 2],
                        op=ALU.add,
                    )
                    nc.vector.tensor_reduce(
                        out=R[:, ks],
                        in_=quar[:],
                        axis=mybir.AxisListType.X,
                        op=ALU.add,
                    )
                E = work.tile([TILE, K], f32, tag="E")
                den = work.tile([TILE, 1], f32, tag="den")
                nc.scalar.activation(E[:], R[:], AF.Exp, accum_out=den[:])
                ld = work.tile([TILE, 1], f32, tag="ld")
                nc.scalar.activation(ld[:], den[:], AF.Ln)
                nld = work.tile([TILE, 1], f32, tag="nld")
                nc.vector.tensor_scalar_mul(nld[:], ld[:], -1.0)
                # softplus(pos - ln(den)) == log1p(exp(pos)/den), decomposed
                # as Exp(pos + (-ln den)) then Ln(1 + t) (Softplus LUT is
                # not available in CoreSim).
                tt = work.tile([TILE, P], f32, tag="tt")
                nc.scalar.activation(tt[:], R[:, 0:P], AF.Exp, bias=nld[:])
                sp = work.tile([TILE, P], f32, tag="sp")
                nc.scalar.activation(
                    sp[:], tt[:], AF.Ln,
                    bias=1.0, accum_out=tlosses[:, t : t + 1],
                )

            # ---- phase 3: reduce to scalar ----
            cs = cpool.tile([TILE, 1], f32)
            nc.vector.tensor_reduce(
                out=cs[:], in_=tlosses[:], axis=mybir.AxisListType.X, op=ALU.add
            )
            ps = psum.tile([1, 1], f32)
            nc.tensor.matmul(ps[:], lhsT=ones[:], rhs=cs[:], start=True, stop=True)
            so = cpool.tile([1, 1], f32)
            nc.vector.tensor_copy(so[:], ps[:])
            nc.sync.dma_start(out=out[:], in_=so[:])

    nc.compile()
    _CACHE["nc"] = nc
    return nc


def kernel(features, positive_index, negative_index):
    from concourse.bass_utils import run_bass_kernel_spmd

    nc = build_program()

    feats = np.ascontiguousarray(np.asarray(features, dtype=np.float32))
    idx = np.concatenate(
        [np.asarray(positive_index), np.asarray(negative_index)], axis=2
    ).astype(np.int16)                      # [B, N, K], values < 4096

    # per (tile, call): k-major flat order (i = k*128 + n -> partition n,
    # slot k), wrapped 16-way and replicated across the 8 partition groups.
    idx_t = idx.reshape(B, NT, TILE, NCALL, KC)      # [B, t, n, c, kc]
    flat = idx_t.transpose(0, 1, 3, 4, 2).reshape(B, NT, NCALL, KC * TILE)
    wrapped = flat.reshape(B, NT, NCALL, KC * TILE // 16, 16).transpose(
        0, 1, 2, 4, 3
    )                                                # [B, t, c, 16, s]
    idxw = np.ascontiguousarray(
        np.tile(wrapped, (1, 1, 1, 8, 1)).astype(np.int16)
    )                                                # [B, NT, NCALL, 128, s]

    core_ids = list(range(B))
    in_maps = [
        {"features": feats[b], "idxw": idxw[b]}
        for b in range(B)
    ]

    import os

    trace = bool(int(os.environ.get("BCL_TRACE", "0")))
    res = run_bass_kernel_spmd(nc, in_maps, core_ids, trace=trace)
    _CACHE["last_run"] = res

    s = np.array([res.results[b]["out"][0, 0] for b in range(B)], dtype=np.float64)
    loss = (-s / (P * N)).mean()
    return np.float32(loss)
